# revision 6
# baseline (speedup 1.0000x reference)
"""MLA (DeepSeek-style multi-head latent attention) forward pass on 8 trn2 cores.

Sharding: tensor-parallel over heads (16 heads -> 2 per core). LoRA-A
projections are replicated; o_proj is input-split on the head dim and the
partial outputs are reduced on the host (the unshard step for this TP layout).

On-device layout: activations are kept transposed [feature, seq] so that every
matmul chains without transposes (contraction dim = partition dim). The V
projection swaps matmul operand roles to produce v in natural [seq, vdim]
orientation. Softmax runs over the partition (key) axis: exp via ScalarE, the
denominator via a ones-row matmul, and the broadcast of per-column scalars
across partitions via a K=1 matmul with a ones column. RoPE's rotate-half is a
PE matmul against a constant signed-permutation matrix (engines cannot move
data across partitions). All matmul operands are float32r (FP22 truncated
multiplies at full PE speed for free dims >= 256).
"""
import numpy as np

import concourse.bass as bass
import concourse.tile as tile
from concourse import bacc, mybir
from concourse.bass_utils import run_bass_kernel_spmd

F32 = mybir.dt.float32
F32R = mybir.dt.float32r

HIDDEN = 2048
S = 2048
NUM_HEADS = 16
Q_LORA = 1536
KV_LORA = 512
NOPE = 128
ROPE = 64
VD = 128
QD = NOPE + ROPE            # 192
SCALE = QD ** -0.5
EPS = 1e-6
ROPE_THETA = 10000.0

NCORES = 8
HPC = NUM_HEADS // NCORES   # heads per core = 2
SB = 512                    # seq block
NSB = S // SB               # 4
KT = HIDDEN // 128          # 16 k-tiles of hidden
QLT = Q_LORA // 128         # 12 tiles of q_latent
CT = KV_LORA // 128         # 4 tiles of compressed kv

_CACHE = {}
LAST_RESULT = None


def _build_program():
    nc = bacc.Bacc("TRN2", target_bir_lowering=False, debug=False,
                   num_devices=NCORES)
    d_xt = nc.dram_tensor("xt", [HIDDEN, S], F32R, kind="ExternalInput").ap()
    d_wqa = nc.dram_tensor("wqa_t", [HIDDEN, Q_LORA], F32R, kind="ExternalInput").ap()
    d_wkva = nc.dram_tensor("wkva_t", [HIDDEN, KV_LORA + ROPE], F32R, kind="ExternalInput").ap()
    d_wqb = nc.dram_tensor("wqb_t", [Q_LORA, HPC * QD], F32R, kind="ExternalInput").ap()
    d_wk = nc.dram_tensor("wk_t", [KV_LORA, HPC * NOPE], F32R, kind="ExternalInput").ap()
    d_wv = nc.dram_tensor("wv_t", [KV_LORA, HPC * VD], F32R, kind="ExternalInput").ap()
    d_wo = nc.dram_tensor("wo_t", [HPC * VD, HIDDEN], F32R, kind="ExternalInput").ap()
    d_cos = nc.dram_tensor("cosd", [128, S], F32R, kind="ExternalInput").ap()
    d_sin = nc.dram_tensor("sind", [128, S], F32R, kind="ExternalInput").ap()
    d_msk = nc.dram_tensor("mask", [128, 4, SB], F32R, kind="ExternalInput").ap()
    d_ones = nc.dram_tensor("ones", [128, 128], F32R, kind="ExternalInput").ap()
    d_rotq = nc.dram_tensor("rotq", [128, 128], F32R, kind="ExternalInput").ap()
    d_dupx = nc.dram_tensor("dupx", [64, 128], F32R, kind="ExternalInput").ap()
    d_duprot = nc.dram_tensor("duprot", [64, 128], F32R, kind="ExternalInput").ap()
    d_out = nc.dram_tensor("out", [S, HIDDEN], F32, kind="ExternalOutput").ap()

    with tile.TileContext(nc) as tc:
        _mla(tc, d_xt, d_wqa, d_wkva, d_wqb, d_wk, d_wv, d_wo, d_cos, d_sin,
             d_msk, d_ones, d_rotq, d_dupx, d_duprot, d_out)
    nc.compile()
    return nc


def _mla(tc, d_xt, d_wqa, d_wkva, d_wqb, d_wk, d_wv, d_wo, d_cos, d_sin,
         d_msk, d_ones, d_rotq, d_dupx, d_duprot, d_out):
    nc = tc.nc
    Exp = mybir.ActivationFunctionType.Exp
    Sqrt = mybir.ActivationFunctionType.Sqrt

    with nc.allow_low_precision(reason="fp32r pipeline: matmul operands are "
                                "deliberately rounded to fp22"), \
         tc.tile_pool(name="pconst", bufs=1) as pc, \
         tc.tile_pool(name="pqkv", bufs=1) as pqkv:
        # constants / small weights, resident for the whole kernel
        ones = pc.tile([128, 128], F32R)
        nc.sync.dma_start(out=ones, in_=d_ones)
        wqb = pc.tile([128, QLT, HPC * QD], F32R)
        nc.sync.dma_start(out=wqb, in_=d_wqb.rearrange("(t p) f -> p t f", p=128))
        wk = pc.tile([128, CT, HPC * NOPE], F32R)
        nc.sync.dma_start(out=wk, in_=d_wk.rearrange("(t p) f -> p t f", p=128))
        wv = pc.tile([128, CT, HPC * VD], F32R)
        nc.sync.dma_start(out=wv, in_=d_wv.rearrange("(t p) f -> p t f", p=128))
        rotq = pc.tile([128, 128], F32R)
        nc.sync.dma_start(out=rotq, in_=d_rotq)
        dupx = pc.tile([64, 128], F32R)
        nc.sync.dma_start(out=dupx, in_=d_dupx)
        duprot = pc.tile([64, 128], F32R)
        nc.sync.dma_start(out=duprot, in_=d_duprot)
        eps1 = pc.tile([1, 1], F32)
        nc.vector.memset(eps1, EPS)

        # persistent per-head tensors (feature-on-partition, full S)
        qn = [pqkv.tile([128, S], F32R, tag=f"qn{h}", name=f"qn{h}") for h in range(HPC)]
        kn = [pqkv.tile([128, S], F32R, tag=f"kn{h}", name=f"kn{h}") for h in range(HPC)]
        qpe = pqkv.tile([128, S], F32R, tag="qpe")    # rows 0-63 h0, 64-127 h1
        kpd = pqkv.tile([128, S], F32R, tag="kpd")    # rope(k_pe) duplicated
        vst = pqkv.tile([128, S // 128, HPC * VD], F32R, tag="vst")
        ao = [pqkv.tile([128, S], F32R, tag=f"ao{h}", name=f"ao{h}")
              for h in range(HPC)]

        # ---------------- stage A: projections, per seq block ----------------
        with tc.tile_pool(name="pcs", bufs=1) as pcs, \
             tc.tile_pool(name="pxt", bufs=16) as pxt, \
             tc.tile_pool(name="pwstr", bufs=3) as pwstr, \
             tc.tile_pool(name="pql", bufs=3) as pql, \
             tc.tile_pool(name="pckv", bufs=6) as pckv, \
             tc.tile_pool(name="psq", bufs=2) as psq, \
             tc.tile_pool(name="pmisc", bufs=2) as pmisc, \
             tc.tile_pool(name="pstatS", bufs=1) as pstatS, \
             tc.tile_pool(name="ppacc", bufs=3, space="PSUM") as ppacc, \
             tc.tile_pool(name="ppstat", bufs=2, space="PSUM") as ppstat, \
             tc.tile_pool(name="ppmt", bufs=3, space="PSUM") as ppmt:
            for b in range(NSB):
                cols = bass.ts(b, SB)
                cosd = pcs.tile([128, SB], F32R, tag="cos", bufs=2)
                nc.sync.dma_start(out=cosd, in_=d_cos[:, cols])
                sind = pcs.tile([128, SB], F32R, tag="sin", bufs=2)
                nc.sync.dma_start(out=sind, in_=d_sin[:, cols])
                xt = []
                for k in range(KT):
                    t = pxt.tile([128, SB], F32R, tag="xt")
                    nc.sync.dma_start(out=t, in_=d_xt[k * 128:(k + 1) * 128, cols])
                    xt.append(t)

                # --- KV LoRA-A: ckv rows [512 c | 64 pe] ---
                ckv = []
                p_cs = ppstat.tile([1, SB], F32, tag="stat")
                for m in range(CT + 1):
                    mw = 128 if m < CT else ROPE
                    wstr = [pwstr.tile([128, KT // 2, 128], F32R, tag="wstr",
                                       name=f"wkva_{m}_{hh}") for hh in range(2)]
                    for hh in range(2):
                        nc.sync.dma_start(
                            out=wstr[hh][:, :, :mw],
                            in_=d_wkva[hh * 1024:(hh + 1) * 1024,
                                       m * 128:m * 128 + mw].rearrange(
                                "(t p) f -> p t f", p=128))
                    p_a = ppacc.tile([128, SB], F32, tag="acc")
                    for k in range(KT):
                        nc.tensor.matmul(p_a[:mw, :], wstr[k // 8][:, k % 8, :mw],
                                         xt[k], start=(k == 0), stop=(k == KT - 1))
                    t = pckv.tile([mw, SB], F32R, tag="ckv")
                    nc.any.tensor_copy(t[:], p_a[:mw, :])
                    ckv.append(t)
                    if m < CT:
                        sq = psq.tile([128, SB], F32R, tag="sq")
                        nc.vector.tensor_mul(sq[:], t[:], t[:])
                        nc.tensor.matmul(p_cs[:], ones[:, 0:1], sq[:],
                                         start=(m == 0), stop=(m == CT - 1))
                # inv rms of compressed kv, replicated across partitions
                cs_s = pstatS.tile([1, SB], F32R, tag="s1")
                nc.scalar.activation(cs_s[:], p_cs[:], Sqrt,
                                     scale=1.0 / KV_LORA, bias=eps1[:])
                p_bc = ppacc.tile([128, SB], F32, tag="acc")
                nc.tensor.matmul(p_bc[:], ones[0:1, :], cs_s[:],
                                 start=True, stop=True)
                invc = pmisc.tile([128, SB], F32R, tag="invc")
                nc.vector.reciprocal(invc[:], p_bc[:])
                for m in range(CT):
                    nc.vector.tensor_mul(ckv[m][:], ckv[m][:], invc[:])

                # --- kv_b: k_nope per head ---
                for h in range(HPC):
                    p_kv = ppmt.tile([128, SB], F32, tag="mt")
                    for k in range(CT):
                        nc.tensor.matmul(p_kv[:], wk[:, k, h * NOPE:(h + 1) * NOPE],
                                         ckv[k][:], start=(k == 0), stop=(k == CT - 1))
                    nc.any.tensor_copy(kn[h][:, cols], p_kv[:])
                # --- v in natural [s, vdim] orientation (swapped operands) ---
                for t4 in range(SB // 128):
                    p_v = ppmt.tile([128, SB], F32, tag="mt")
                    for k in range(CT):
                        nc.tensor.matmul(p_v[:, :HPC * VD],
                                         ckv[k][:, t4 * 128:(t4 + 1) * 128],
                                         wv[:, k, :], start=(k == 0), stop=(k == CT - 1))
                    nc.any.tensor_copy(vst[:, b * (SB // 128) + t4, :],
                                       p_v[:, :HPC * VD])

                # --- k_pe rope + duplicate to both 64-row halves ---
                kpe_raw = ckv[CT]                       # [64, SB]
                p_x = ppacc.tile([128, SB], F32, tag="acc")
                nc.tensor.matmul(p_x[:], dupx[:], kpe_raw[:], start=True, stop=True)
                p_r = ppmt.tile([128, SB], F32, tag="mt")
                nc.tensor.matmul(p_r[:], duprot[:], kpe_raw[:], start=True, stop=True)
                t1 = pmisc.tile([128, SB], F32R, tag="t1")
                nc.vector.tensor_mul(t1[:], p_x[:], cosd[:])
                t2 = pmisc.tile([128, SB], F32R, tag="t2")
                nc.vector.tensor_mul(t2[:], p_r[:], sind[:])
                nc.vector.tensor_add(kpd[:, cols], t1[:], t2[:])

                # --- Q LoRA-A + q_b fused over latent tiles ---
                p_q = [ppmt.tile([128, SB], F32, tag="mt", name=f"p_q{_i}") for _i in range(3)]
                p_qs = ppstat.tile([1, SB], F32, tag="stat")
                for k in range(QLT):
                    wstr = [pwstr.tile([128, KT // 2, 128], F32R, tag="wstr",
                                       name=f"wqa_{k}_{hh}") for hh in range(2)]
                    for hh in range(2):
                        nc.sync.dma_start(
                            out=wstr[hh],
                            in_=d_wqa[hh * 1024:(hh + 1) * 1024,
                                      k * 128:(k + 1) * 128].rearrange(
                                "(t p) f -> p t f", p=128))
                    p_a = ppacc.tile([128, SB], F32, tag="acc")
                    for kk in range(KT):
                        nc.tensor.matmul(p_a[:], wstr[kk // 8][:, kk % 8, :],
                                         xt[kk], start=(kk == 0), stop=(kk == KT - 1))
                    ql = pql.tile([128, SB], F32R, tag="ql")
                    nc.any.tensor_copy(ql[:], p_a[:])
                    sq = psq.tile([128, SB], F32R, tag="sq")
                    nc.vector.tensor_mul(sq[:], ql[:], ql[:])
                    nc.tensor.matmul(p_qs[:], ones[:, 0:1], sq[:],
                                     start=(k == 0), stop=(k == QLT - 1))
                    for mt in range(3):
                        nc.tensor.matmul(p_q[mt][:], wqb[:, k, mt * 128:(mt + 1) * 128],
                                         ql[:], start=(k == 0), stop=(k == QLT - 1))
                qs_s = pstatS.tile([1, SB], F32R, tag="s2")
                nc.scalar.activation(qs_s[:], p_qs[:], Sqrt,
                                     scale=1.0 / Q_LORA, bias=eps1[:])
                p_bc2 = ppacc.tile([128, SB], F32, tag="acc")
                nc.tensor.matmul(p_bc2[:], ones[0:1, :], qs_s[:],
                                 start=True, stop=True)
                invq = pmisc.tile([128, SB], F32R, tag="invq")
                nc.vector.reciprocal(invq[:], p_bc2[:])
                nc.vector.tensor_mul(qn[0][:, cols], p_q[0][:], invq[:])
                nc.vector.tensor_mul(qn[1][:, cols], p_q[1][:], invq[:])
                qpe_raw = psq.tile([128, SB], F32R, tag="sq", name="qpe_raw")
                nc.vector.tensor_mul(qpe_raw[:], p_q[2][:], invq[:])
                # q_pe rope (both heads packed in 64-row halves)
                p_rq = ppacc.tile([128, SB], F32, tag="acc")
                nc.tensor.matmul(p_rq[:], rotq[:], qpe_raw[:], start=True, stop=True)
                t1q = pmisc.tile([128, SB], F32R, tag="t1")
                nc.vector.tensor_mul(t1q[:], qpe_raw[:], cosd[:])
                t2q = pmisc.tile([128, SB], F32R, tag="t2")
                nc.vector.tensor_mul(t2q[:], p_rq[:], sind[:])
                nc.vector.tensor_add(qpe[:, cols], t1q[:], t2q[:])

        # ---------------- stage B: causal attention per head ----------------
        with tc.tile_pool(name="pbm", bufs=1) as pbm, \
             tc.tile_pool(name="pexp", bufs=3) as pexp, \
             tc.tile_pool(name="pbn", bufs=2) as pbn, \
             tc.tile_pool(name="ppS", bufs=2, space="PSUM") as ppS, \
             tc.tile_pool(name="ppO", bufs=2, space="PSUM") as ppO, \
             tc.tile_pool(name="ppD", bufs=2, space="PSUM") as ppD, \
             tc.tile_pool(name="ppB", bufs=2, space="PSUM") as ppB:
            msk = pbm.tile([128, 4, SB], F32R)
            nc.sync.dma_start(out=msk, in_=d_msk)
            for h in range(HPC):
                hp = slice(64 * h, 64 * h + 64)
                for qb in range(NSB):
                    qcols = bass.ts(qb, SB)
                    nk = 4 * (qb + 1)
                    p_o = ppO.tile([128, SB], F32, tag="o")
                    p_d = ppD.tile([1, SB], F32, tag="d")
                    for ik in range(nk):
                        kc = slice(ik * 128, (ik + 1) * 128)
                        p_s = ppS.tile([128, SB], F32, tag="s")
                        nc.tensor.matmul(p_s[:], kn[h][:, kc], qn[h][:, qcols],
                                         start=True, stop=False)
                        nc.tensor.matmul(p_s[:], kpd[hp, kc], qpe[hp, qcols],
                                         start=False, stop=True)
                        e = pexp.tile([128, SB], F32R, tag="e")
                        nc.scalar.activation(e[:], p_s[:], Exp, scale=SCALE)
                        r = ik - 4 * qb
                        if r >= 0:
                            nc.vector.tensor_mul(e[:], e[:], msk[:, r, :])
                        nc.tensor.matmul(p_o[:], vst[:, ik, h * VD:(h + 1) * VD],
                                         e[:], start=(ik == 0), stop=(ik == nk - 1))
                        nc.tensor.matmul(p_d[:], ones[:, 0:1], e[:],
                                         start=(ik == 0), stop=(ik == nk - 1))
                    den = pbn.tile([1, SB], F32R, tag="den")
                    nc.any.tensor_copy(den[:], p_d[:])
                    p_bc = ppB.tile([128, SB], F32, tag="bc")
                    nc.tensor.matmul(p_bc[:], ones[0:1, :], den[:],
                                     start=True, stop=True)
                    rec = pbn.tile([128, SB], F32, tag="rec")
                    nc.vector.reciprocal(rec[:], p_bc[:])
                    nc.vector.tensor_mul(ao[h][:, qcols], p_o[:], rec[:])

        # ---------------- stage C: output projection ----------------
        with tc.tile_pool(name="pwo", bufs=1) as pwo, \
             tc.tile_pool(name="pout", bufs=3) as pout, \
             tc.tile_pool(name="ppC", bufs=4, space="PSUM") as ppC:
            wo = pwo.tile([128, HPC, HIDDEN], F32R)
            nc.sync.dma_start(out=wo, in_=d_wo.rearrange("(t p) f -> p t f", p=128))
            for st in range(S // 128):
                sc = slice(st * 128, (st + 1) * 128)
                for nb in range(HIDDEN // SB):
                    ncols = bass.ts(nb, SB)
                    p_c = ppC.tile([128, SB], F32, tag="c")
                    for h in range(HPC):
                        nc.tensor.matmul(p_c[:], ao[h][:, sc], wo[:, h, ncols],
                                         start=(h == 0), stop=(h == HPC - 1))
                    ot = pout.tile([128, SB], F32, tag="ot")
                    nc.any.tensor_copy(ot[:], p_c[:])
                    nc.sync.dma_start(out=d_out[sc, ncols], in_=ot[:])


def _host_constants():
    inv_freq = 1.0 / (ROPE_THETA ** (np.arange(0, ROPE, 2, dtype=np.float32) / ROPE))
    t = np.arange(S, dtype=np.float32)
    freqs = np.outer(t, inv_freq)
    emb = np.concatenate([freqs, freqs], -1)          # [S, 64]
    cos, sin = np.cos(emb), np.sin(emb)
    cosd = np.concatenate([cos.T, cos.T], 0).astype(np.float32)   # [128, S]
    sind = np.concatenate([sin.T, sin.T], 0).astype(np.float32)

    msk = np.zeros((128, 4, SB), np.float32)
    for r in range(4):
        for p in range(128):
            k_idx = p + 128 * r
            if k_idx < SB:
                msk[p, r, k_idx:] = 1.0               # keep where k <= q
    onesm = np.ones((128, 128), np.float32)

    # rotate-half as matrices: rot = P @ x, per 64-row block
    Q = np.zeros((64, 64), np.float32)
    for i in range(32):
        Q[i, i + 32] = -1.0
        Q[i + 32, i] = 1.0
    P = np.zeros((128, 128), np.float32)
    P[:64, :64] = Q
    P[64:, 64:] = Q
    rotq = P.T.copy()                                  # lhsT
    D = np.concatenate([np.eye(64, dtype=np.float32)] * 2, 0)   # [128, 64]
    dupx = D.T.copy()                                  # [64, 128]
    duprot = np.concatenate([Q, Q], 0).T.copy()        # [64, 128]
    return cosd, sind, msk, onesm, rotq, dupx, duprot


def kernel(hidden_states, w_q_a, q_a_weight, w_q_b, w_kv_a, kv_a_weight,
           w_kv_b, w_o):
    global LAST_RESULT
    if "nc" not in _CACHE:
        _CACHE["nc"] = _build_program()
    nc = _CACHE["nc"]

    x = np.asarray(hidden_states, np.float32)[0]       # [S, 2048]
    xt = np.ascontiguousarray(x.T)
    wqa_t = np.ascontiguousarray(np.asarray(w_q_a, np.float32).T)
    wkva_t = np.ascontiguousarray(np.asarray(w_kv_a, np.float32).T)
    wqb_eff = np.asarray(w_q_b, np.float32) * np.asarray(q_a_weight, np.float32)[None, :]
    wkvb_eff = np.asarray(w_kv_b, np.float32) * np.asarray(kv_a_weight, np.float32)[None, :]
    won = np.asarray(w_o, np.float32)

    cosd, sind, msk, onesm, rotq, dupx, duprot = _host_constants()
    shared = {"xt": xt, "wqa_t": wqa_t, "wkva_t": wkva_t, "cosd": cosd,
              "sind": sind, "mask": msk, "ones": onesm, "rotq": rotq,
              "dupx": dupx, "duprot": duprot}

    in_maps = []
    for c in range(NCORES):
        h0, h1 = HPC * c, HPC * c + 1
        # wqb_t cols: [h0 nope | h1 nope | h0 pe | h1 pe]
        cols = []
        for h in (h0, h1):
            cols.append(wqb_eff[h * QD:h * QD + NOPE])         # [128, 1536]
        for h in (h0, h1):
            cols.append(wqb_eff[h * QD + NOPE:(h + 1) * QD])   # [64, 1536]
        wqb_t = np.ascontiguousarray(np.concatenate(cols, 0).T)  # [1536, 384]
        wk_t = np.ascontiguousarray(np.concatenate(
            [wkvb_eff[h * (NOPE + VD):h * (NOPE + VD) + NOPE] for h in (h0, h1)],
            0).T)                                               # [512, 256]
        wv_t = np.ascontiguousarray(np.concatenate(
            [wkvb_eff[h * (NOPE + VD) + NOPE:(h + 1) * (NOPE + VD)] for h in (h0, h1)],
            0).T)                                               # [512, 256]
        wo_t = np.ascontiguousarray(np.concatenate(
            [won[:, h * VD:(h + 1) * VD] for h in (h0, h1)], 1).T)  # [256, 2048]
        im = dict(shared)
        im.update({"wqb_t": wqb_t, "wk_t": wk_t, "wv_t": wv_t, "wo_t": wo_t})
        in_maps.append(im)

    res = run_bass_kernel_spmd(nc, in_maps, list(range(NCORES)))
    LAST_RESULT = res
    out = np.zeros((S, HIDDEN), np.float32)
    for c in range(NCORES):
        out += res.results[c]["out"]
    return out.reshape(1, S, HIDDEN)


# revision 9
# speedup vs baseline: 1.1376x; 1.1376x over previous
"""MLA (DeepSeek-style multi-head latent attention) forward pass on 8 trn2 cores.

Sharding: tensor-parallel over heads (16 heads -> 2 per core). LoRA-A
projections are replicated; o_proj is input-split on the head dim and the
partial outputs are reduced on the host (the unshard step for this TP layout).

On-device layout: activations are kept transposed [feature, seq] so that every
matmul chains without transposes (contraction dim = partition dim). The V
projection swaps matmul operand roles to produce v in natural [seq, vdim]
orientation. Softmax runs over the partition (key) axis: exp via ScalarE, the
denominator via a ones-row matmul, and the broadcast of per-column scalars
across partitions via a K=1 matmul with a ones column. RoPE's rotate-half is a
PE matmul against a constant signed-permutation matrix (engines cannot move
data across partitions). All matmul operands are float32r (FP22 truncated
multiplies at full PE speed for free dims >= 256).
"""
import numpy as np

import concourse.bass as bass
import concourse.tile as tile
from concourse import bacc, mybir
from concourse.bass_utils import run_bass_kernel_spmd

F32 = mybir.dt.float32
F32R = mybir.dt.float32r

HIDDEN = 2048
S = 2048
NUM_HEADS = 16
Q_LORA = 1536
KV_LORA = 512
NOPE = 128
ROPE = 64
VD = 128
QD = NOPE + ROPE            # 192
SCALE = QD ** -0.5
EPS = 1e-6
ROPE_THETA = 10000.0

NCORES = 8
HPC = NUM_HEADS // NCORES   # heads per core = 2
SB = 512                    # seq block
NSB = S // SB               # 4
KT = HIDDEN // 128          # 16 k-tiles of hidden
QLT = Q_LORA // 128         # 12 tiles of q_latent
CT = KV_LORA // 128         # 4 tiles of compressed kv

_CACHE = {}
LAST_RESULT = None


def _build_program():
    nc = bacc.Bacc("TRN2", target_bir_lowering=False, debug=False,
                   num_devices=NCORES)
    d_xt = nc.dram_tensor("xt", [HIDDEN, S], F32R, kind="ExternalInput").ap()
    d_wqa = nc.dram_tensor("wqa_t", [HIDDEN, Q_LORA], F32R, kind="ExternalInput").ap()
    d_wkva = nc.dram_tensor("wkva_t", [HIDDEN, KV_LORA + ROPE], F32R, kind="ExternalInput").ap()
    d_wqb = nc.dram_tensor("wqb_t", [Q_LORA, HPC * QD], F32R, kind="ExternalInput").ap()
    d_wk = nc.dram_tensor("wk_t", [KV_LORA, HPC * NOPE], F32R, kind="ExternalInput").ap()
    d_wv = nc.dram_tensor("wv_t", [KV_LORA, HPC * VD], F32R, kind="ExternalInput").ap()
    d_wo = nc.dram_tensor("wo_t", [HPC * VD, HIDDEN], F32R, kind="ExternalInput").ap()
    d_cos = nc.dram_tensor("cosd", [128, S], F32R, kind="ExternalInput").ap()
    d_sin = nc.dram_tensor("sind", [128, S], F32R, kind="ExternalInput").ap()
    d_msk = nc.dram_tensor("mask", [128, 4, SB], F32R, kind="ExternalInput").ap()
    d_ones = nc.dram_tensor("ones", [128, 128], F32R, kind="ExternalInput").ap()
    d_rotq = nc.dram_tensor("rotq", [128, 128], F32R, kind="ExternalInput").ap()
    d_dupx = nc.dram_tensor("dupx", [64, 128], F32R, kind="ExternalInput").ap()
    d_duprot = nc.dram_tensor("duprot", [64, 128], F32R, kind="ExternalInput").ap()
    d_out = nc.dram_tensor("out", [S, HIDDEN], F32, kind="ExternalOutput").ap()

    with tile.TileContext(nc) as tc:
        _mla(tc, d_xt, d_wqa, d_wkva, d_wqb, d_wk, d_wv, d_wo, d_cos, d_sin,
             d_msk, d_ones, d_rotq, d_dupx, d_duprot, d_out)
    nc.compile()
    return nc


def _mla(tc, d_xt, d_wqa, d_wkva, d_wqb, d_wk, d_wv, d_wo, d_cos, d_sin,
         d_msk, d_ones, d_rotq, d_dupx, d_duprot, d_out):
    nc = tc.nc
    Exp = mybir.ActivationFunctionType.Exp
    Sqrt = mybir.ActivationFunctionType.Sqrt

    with nc.allow_low_precision(reason="fp32r pipeline: matmul operands are "
                                "deliberately rounded to fp22"), \
         tc.tile_pool(name="pconst", bufs=1) as pc, \
         tc.tile_pool(name="pqkv", bufs=1) as pqkv:
        # constants / small weights, resident for the whole kernel
        ones = pc.tile([128, 128], F32R)
        nc.sync.dma_start(out=ones, in_=d_ones)
        wqb = pc.tile([128, QLT, HPC * QD], F32R)
        wk = pc.tile([128, CT, HPC * NOPE], F32R)
        wv = pc.tile([128, CT, HPC * VD], F32R)
        rotq = pc.tile([128, 128], F32R)
        nc.sync.dma_start(out=rotq, in_=d_rotq)
        dupx = pc.tile([64, 128], F32R)
        nc.sync.dma_start(out=dupx, in_=d_dupx)
        duprot = pc.tile([64, 128], F32R)
        nc.sync.dma_start(out=duprot, in_=d_duprot)
        eps1 = pc.tile([1, 1], F32)
        nc.vector.memset(eps1, EPS)

        # persistent per-head tensors (feature-on-partition, full S)
        qn = [pqkv.tile([128, S], F32R, tag=f"qn{h}", name=f"qn{h}") for h in range(HPC)]
        kn = [pqkv.tile([128, S], F32R, tag=f"kn{h}", name=f"kn{h}") for h in range(HPC)]
        qpe = pqkv.tile([128, S], F32R, tag="qpe")    # rows 0-63 h0, 64-127 h1
        kpd = pqkv.tile([128, S], F32R, tag="kpd")    # rope(k_pe) duplicated
        vst = pqkv.tile([128, S // 128, HPC * VD], F32R, tag="vst")
        ao = [pqkv.tile([128, S], F32R, tag=f"ao{h}", name=f"ao{h}")
              for h in range(HPC)]

        # ---------------- stage A: projections, per seq block ----------------
        with tc.tile_pool(name="pcs", bufs=1) as pcs, \
             tc.tile_pool(name="pxt", bufs=19) as pxt, \
             tc.tile_pool(name="pwstr", bufs=3) as pwstr, \
             tc.tile_pool(name="pql", bufs=3) as pql, \
             tc.tile_pool(name="pckv", bufs=5) as pckv, \
             tc.tile_pool(name="psq", bufs=2) as psq, \
             tc.tile_pool(name="pmisc", bufs=2) as pmisc, \
             tc.tile_pool(name="pstatS", bufs=1) as pstatS, \
             tc.tile_pool(name="ppacc", bufs=3, space="PSUM") as ppacc, \
             tc.tile_pool(name="ppstat", bufs=2, space="PSUM") as ppstat, \
             tc.tile_pool(name="ppmt", bufs=3, space="PSUM") as ppmt:
            for b in range(NSB):
                cols = bass.ts(b, SB)
                cosd = pcs.tile([128, SB], F32R, tag="cos", bufs=2)
                nc.sync.dma_start(out=cosd, in_=d_cos[:, cols])
                sind = pcs.tile([128, SB], F32R, tag="sin", bufs=2)
                nc.sync.dma_start(out=sind, in_=d_sin[:, cols])
                xt = []
                for k in range(KT):
                    t = pxt.tile([128, SB], F32R, tag="xt")
                    nc.sync.dma_start(out=t, in_=d_xt[k * 128:(k + 1) * 128, cols])
                    xt.append(t)

                if b == 0:
                    # deferred resident-weight loads: issued after block 0's
                    # xt/lora-weight DMAs so they don't delay the first matmuls
                    nc.sync.dma_start(out=wk, in_=d_wk.rearrange("(t p) f -> p t f", p=128))
                    nc.sync.dma_start(out=wv, in_=d_wv.rearrange("(t p) f -> p t f", p=128))
                    nc.sync.dma_start(out=wqb, in_=d_wqb.rearrange("(t p) f -> p t f", p=128))
                # --- KV LoRA-A: ckv rows [512 c | 64 pe] ---
                ckv = []
                p_cs = ppstat.tile([1, SB], F32, tag="stat")
                for m in range(CT + 1):
                    mw = 128 if m < CT else ROPE
                    wstr = [pwstr.tile([128, KT // 2, 128], F32R, tag="wstr",
                                       name=f"wkva_{m}_{hh}") for hh in range(2)]
                    for hh in range(2):
                        nc.sync.dma_start(
                            out=wstr[hh][:, :, :mw],
                            in_=d_wkva[hh * 1024:(hh + 1) * 1024,
                                       m * 128:m * 128 + mw].rearrange(
                                "(t p) f -> p t f", p=128))
                    p_a = ppacc.tile([128, SB], F32, tag="acc")
                    for k in range(KT):
                        nc.tensor.matmul(p_a[:mw, :], wstr[k // 8][:, k % 8, :mw],
                                         xt[k], start=(k == 0), stop=(k == KT - 1))
                    t = pckv.tile([mw, SB], F32R, tag="ckv")
                    nc.any.tensor_copy(t[:], p_a[:mw, :])
                    ckv.append(t)
                    if m < CT:
                        sq = psq.tile([128, SB], F32R, tag="sq")
                        nc.vector.tensor_mul(sq[:], t[:], t[:])
                        nc.tensor.matmul(p_cs[:], ones[:, 0:1], sq[:],
                                         start=(m == 0), stop=(m == CT - 1))
                # inv rms of compressed kv, replicated across partitions
                cs_s = pstatS.tile([1, SB], F32R, tag="s1")
                nc.scalar.activation(cs_s[:], p_cs[:], Sqrt,
                                     scale=1.0 / KV_LORA, bias=eps1[:])
                p_bc = ppacc.tile([128, SB], F32, tag="acc")
                nc.tensor.matmul(p_bc[:], ones[0:1, :], cs_s[:],
                                 start=True, stop=True)
                invc = pmisc.tile([128, SB], F32R, tag="invc")
                nc.vector.reciprocal(invc[:], p_bc[:])
                for m in range(CT):
                    nc.vector.tensor_mul(ckv[m][:], ckv[m][:], invc[:])

                # --- kv_b: k_nope per head ---
                for h in range(HPC):
                    p_kv = ppmt.tile([128, SB], F32, tag="mt")
                    for k in range(CT):
                        nc.tensor.matmul(p_kv[:], wk[:, k, h * NOPE:(h + 1) * NOPE],
                                         ckv[k][:], start=(k == 0), stop=(k == CT - 1))
                    nc.any.tensor_copy(kn[h][:, cols], p_kv[:])
                # --- v in natural [s, vdim] orientation (swapped operands) ---
                for t4 in range(SB // 128):
                    p_v = ppmt.tile([128, SB], F32, tag="mt")
                    for k in range(CT):
                        nc.tensor.matmul(p_v[:, :HPC * VD],
                                         ckv[k][:, t4 * 128:(t4 + 1) * 128],
                                         wv[:, k, :], start=(k == 0), stop=(k == CT - 1))
                    nc.any.tensor_copy(vst[:, b * (SB // 128) + t4, :],
                                       p_v[:, :HPC * VD])

                # --- k_pe rope + duplicate to both 64-row halves ---
                kpe_raw = ckv[CT]                       # [64, SB]
                p_x = ppacc.tile([128, SB], F32, tag="acc")
                nc.tensor.matmul(p_x[:], dupx[:], kpe_raw[:], start=True, stop=True)
                p_r = ppmt.tile([128, SB], F32, tag="mt")
                nc.tensor.matmul(p_r[:], duprot[:], kpe_raw[:], start=True, stop=True)
                t1 = pmisc.tile([128, SB], F32R, tag="t1")
                nc.vector.tensor_mul(t1[:], p_x[:], cosd[:])
                t2 = pmisc.tile([128, SB], F32R, tag="t2")
                nc.vector.tensor_mul(t2[:], p_r[:], sind[:])
                nc.vector.tensor_add(kpd[:, cols], t1[:], t2[:])

                # --- Q LoRA-A + q_b fused over latent tiles ---
                p_q = [ppmt.tile([128, SB], F32, tag="mt", name=f"p_q{_i}") for _i in range(3)]
                p_qs = ppstat.tile([1, SB], F32, tag="stat")
                for k in range(QLT):
                    wstr = [pwstr.tile([128, KT // 2, 128], F32R, tag="wstr",
                                       name=f"wqa_{k}_{hh}") for hh in range(2)]
                    for hh in range(2):
                        nc.sync.dma_start(
                            out=wstr[hh],
                            in_=d_wqa[hh * 1024:(hh + 1) * 1024,
                                      k * 128:(k + 1) * 128].rearrange(
                                "(t p) f -> p t f", p=128))
                    p_a = ppacc.tile([128, SB], F32, tag="acc")
                    for kk in range(KT):
                        nc.tensor.matmul(p_a[:], wstr[kk // 8][:, kk % 8, :],
                                         xt[kk], start=(kk == 0), stop=(kk == KT - 1))
                    ql = pql.tile([128, SB], F32R, tag="ql")
                    nc.any.tensor_copy(ql[:], p_a[:])
                    sq = psq.tile([128, SB], F32R, tag="sq")
                    nc.vector.tensor_mul(sq[:], ql[:], ql[:])
                    nc.tensor.matmul(p_qs[:], ones[:, 0:1], sq[:],
                                     start=(k == 0), stop=(k == QLT - 1))
                    for mt in range(3):
                        nc.tensor.matmul(p_q[mt][:], wqb[:, k, mt * 128:(mt + 1) * 128],
                                         ql[:], start=(k == 0), stop=(k == QLT - 1))
                qs_s = pstatS.tile([1, SB], F32R, tag="s2")
                nc.scalar.activation(qs_s[:], p_qs[:], Sqrt,
                                     scale=1.0 / Q_LORA, bias=eps1[:])
                p_bc2 = ppacc.tile([128, SB], F32, tag="acc")
                nc.tensor.matmul(p_bc2[:], ones[0:1, :], qs_s[:],
                                 start=True, stop=True)
                invq = pmisc.tile([128, SB], F32R, tag="invq")
                nc.vector.reciprocal(invq[:], p_bc2[:])
                nc.vector.tensor_mul(qn[0][:, cols], p_q[0][:], invq[:])
                nc.vector.tensor_mul(qn[1][:, cols], p_q[1][:], invq[:])
                qpe_raw = psq.tile([128, SB], F32R, tag="sq", name="qpe_raw")
                nc.vector.tensor_mul(qpe_raw[:], p_q[2][:], invq[:])
                # q_pe rope (both heads packed in 64-row halves)
                p_rq = ppacc.tile([128, SB], F32, tag="acc")
                nc.tensor.matmul(p_rq[:], rotq[:], qpe_raw[:], start=True, stop=True)
                t1q = pmisc.tile([128, SB], F32R, tag="t1")
                nc.vector.tensor_mul(t1q[:], qpe_raw[:], cosd[:])
                t2q = pmisc.tile([128, SB], F32R, tag="t2")
                nc.vector.tensor_mul(t2q[:], p_rq[:], sind[:])
                nc.vector.tensor_add(qpe[:, cols], t1q[:], t2q[:])

        # ------- stage B+C: attention per (block, head) + fused o_proj -------
        # qb-outer so each block's o-projection (output-DMA-bound) overlaps
        # the next block's attention compute.
        with tc.tile_pool(name="pbm", bufs=1) as pbm, \
             tc.tile_pool(name="pexp", bufs=3) as pexp, \
             tc.tile_pool(name="pbn", bufs=2) as pbn, \
             tc.tile_pool(name="pout", bufs=3) as pout, \
             tc.tile_pool(name="ppS", bufs=2, space="PSUM") as ppS, \
             tc.tile_pool(name="ppO", bufs=2, space="PSUM") as ppO, \
             tc.tile_pool(name="ppD", bufs=1, space="PSUM") as ppD, \
             tc.tile_pool(name="ppB", bufs=1, space="PSUM") as ppB, \
             tc.tile_pool(name="ppC", bufs=2, space="PSUM") as ppC:
            msk = pbm.tile([128, 4, SB], F32R)
            nc.sync.dma_start(out=msk, in_=d_msk)
            wo = pbm.tile([128, HPC, HIDDEN], F32R)
            nc.sync.dma_start(out=wo, in_=d_wo.rearrange("(t p) f -> p t f", p=128))
            for qb in range(NSB):
                qcols = bass.ts(qb, SB)
                nk = 4 * (qb + 1)
                for h in range(HPC):
                    hp = slice(64 * h, 64 * h + 64)
                    p_o = ppO.tile([128, SB], F32, tag="o")
                    p_d = ppD.tile([1, SB], F32, tag="d")
                    for ik in range(nk):
                        kc = slice(ik * 128, (ik + 1) * 128)
                        p_s = ppS.tile([128, SB], F32, tag="s")
                        nc.tensor.matmul(p_s[:], kn[h][:, kc], qn[h][:, qcols],
                                         start=True, stop=False)
                        nc.tensor.matmul(p_s[:], kpd[hp, kc], qpe[hp, qcols],
                                         start=False, stop=True)
                        e = pexp.tile([128, SB], F32R, tag="e")
                        nc.scalar.activation(e[:], p_s[:], Exp, scale=SCALE)
                        r = ik - 4 * qb
                        if r >= 0:
                            nc.vector.tensor_mul(e[:], e[:], msk[:, r, :])
                        nc.tensor.matmul(p_o[:], vst[:, ik, h * VD:(h + 1) * VD],
                                         e[:], start=(ik == 0), stop=(ik == nk - 1))
                        nc.tensor.matmul(p_d[:], ones[:, 0:1], e[:],
                                         start=(ik == 0), stop=(ik == nk - 1))
                    den = pbn.tile([1, SB], F32R, tag="den")
                    nc.any.tensor_copy(den[:], p_d[:])
                    p_bc = ppB.tile([128, SB], F32, tag="bc")
                    nc.tensor.matmul(p_bc[:], ones[0:1, :], den[:],
                                     start=True, stop=True)
                    rec = pbn.tile([128, SB], F32, tag="rec")
                    nc.vector.reciprocal(rec[:], p_bc[:])
                    nc.vector.tensor_mul(ao[h][:, qcols], p_o[:], rec[:])
                # o-projection for this block's 4 row-tiles (both heads ready)
                for st in range(qb * (SB // 128), (qb + 1) * (SB // 128)):
                    sc = slice(st * 128, (st + 1) * 128)
                    for nb in range(HIDDEN // SB):
                        ncols = bass.ts(nb, SB)
                        p_c = ppC.tile([128, SB], F32, tag="c")
                        for h in range(HPC):
                            nc.tensor.matmul(p_c[:], ao[h][:, sc], wo[:, h, ncols],
                                             start=(h == 0), stop=(h == HPC - 1))
                        ot = pout.tile([128, SB], F32, tag="ot")
                        nc.any.tensor_copy(ot[:], p_c[:])
                        nc.sync.dma_start(out=d_out[sc, ncols], in_=ot[:])


def _host_constants():
    inv_freq = 1.0 / (ROPE_THETA ** (np.arange(0, ROPE, 2, dtype=np.float32) / ROPE))
    t = np.arange(S, dtype=np.float32)
    freqs = np.outer(t, inv_freq)
    emb = np.concatenate([freqs, freqs], -1)          # [S, 64]
    cos, sin = np.cos(emb), np.sin(emb)
    cosd = np.concatenate([cos.T, cos.T], 0).astype(np.float32)   # [128, S]
    sind = np.concatenate([sin.T, sin.T], 0).astype(np.float32)

    msk = np.zeros((128, 4, SB), np.float32)
    for r in range(4):
        for p in range(128):
            k_idx = p + 128 * r
            if k_idx < SB:
                msk[p, r, k_idx:] = 1.0               # keep where k <= q
    onesm = np.ones((128, 128), np.float32)

    # rotate-half as matrices: rot = P @ x, per 64-row block
    Q = np.zeros((64, 64), np.float32)
    for i in range(32):
        Q[i, i + 32] = -1.0
        Q[i + 32, i] = 1.0
    P = np.zeros((128, 128), np.float32)
    P[:64, :64] = Q
    P[64:, 64:] = Q
    rotq = P.T.copy()                                  # lhsT
    D = np.concatenate([np.eye(64, dtype=np.float32)] * 2, 0)   # [128, 64]
    dupx = D.T.copy()                                  # [64, 128]
    duprot = np.concatenate([Q, Q], 0).T.copy()        # [64, 128]
    return cosd, sind, msk, onesm, rotq, dupx, duprot


def kernel(hidden_states, w_q_a, q_a_weight, w_q_b, w_kv_a, kv_a_weight,
           w_kv_b, w_o):
    global LAST_RESULT
    if "nc" not in _CACHE:
        _CACHE["nc"] = _build_program()
    nc = _CACHE["nc"]

    x = np.asarray(hidden_states, np.float32)[0]       # [S, 2048]
    xt = np.ascontiguousarray(x.T)
    wqa_t = np.ascontiguousarray(np.asarray(w_q_a, np.float32).T)
    wkva_t = np.ascontiguousarray(np.asarray(w_kv_a, np.float32).T)
    wqb_eff = np.asarray(w_q_b, np.float32) * np.asarray(q_a_weight, np.float32)[None, :]
    wkvb_eff = np.asarray(w_kv_b, np.float32) * np.asarray(kv_a_weight, np.float32)[None, :]
    won = np.asarray(w_o, np.float32)

    cosd, sind, msk, onesm, rotq, dupx, duprot = _host_constants()
    shared = {"xt": xt, "wqa_t": wqa_t, "wkva_t": wkva_t, "cosd": cosd,
              "sind": sind, "mask": msk, "ones": onesm, "rotq": rotq,
              "dupx": dupx, "duprot": duprot}

    in_maps = []
    for c in range(NCORES):
        h0, h1 = HPC * c, HPC * c + 1
        # wqb_t cols: [h0 nope | h1 nope | h0 pe | h1 pe]
        cols = []
        for h in (h0, h1):
            cols.append(wqb_eff[h * QD:h * QD + NOPE])         # [128, 1536]
        for h in (h0, h1):
            cols.append(wqb_eff[h * QD + NOPE:(h + 1) * QD])   # [64, 1536]
        wqb_t = np.ascontiguousarray(np.concatenate(cols, 0).T)  # [1536, 384]
        wk_t = np.ascontiguousarray(np.concatenate(
            [wkvb_eff[h * (NOPE + VD):h * (NOPE + VD) + NOPE] for h in (h0, h1)],
            0).T)                                               # [512, 256]
        wv_t = np.ascontiguousarray(np.concatenate(
            [wkvb_eff[h * (NOPE + VD) + NOPE:(h + 1) * (NOPE + VD)] for h in (h0, h1)],
            0).T)                                               # [512, 256]
        wo_t = np.ascontiguousarray(np.concatenate(
            [won[:, h * VD:(h + 1) * VD] for h in (h0, h1)], 1).T)  # [256, 2048]
        im = dict(shared)
        im.update({"wqb_t": wqb_t, "wk_t": wk_t, "wv_t": wv_t, "wo_t": wo_t})
        in_maps.append(im)

    res = run_bass_kernel_spmd(nc, in_maps, list(range(NCORES)))
    LAST_RESULT = res
    out = np.zeros((S, HIDDEN), np.float32)
    for c in range(NCORES):
        out += res.results[c]["out"]
    return out.reshape(1, S, HIDDEN)
